# revision 43
# baseline (speedup 1.0000x reference)
"""Trainium2 Bass kernel for nn_ClusteringLayer (Student-t / vq_codebook).

Math (ALPHA=1): out[n,k] = q_nk / sum_k q_nk,  q = 1/(1 + ||x_n - c_k||^2)
             ||x-c||^2 = xsq + csq - 2 x.c

Sharding: data-parallel over batch dim (8 batches -> 8 NeuronCores); the
(8,32) cluster table is replicated; row-normalization is local per pixel.

Per-core layout (P = 65536 pixels, F = 32 feat, K = 8 clusters):
  pixel n = 8192*g + 2048*v + 16*p + 4*t + b     (g:8, v:4, p:128, t:4, b:4)

  per (g,t):  load xnat[p, 128v+32b+f] = x[n]    (bf16 cast in DMA,
              512B-contiguous reads)
  per (g,t,v): PE-transpose -> psum_xT[32b+f, p] -> ACT copy -> rhs1 (SBUF)
              DVE square -> rhs2 = rhs1^2
  matmuls into one PSUM bank u[128 p, 512 = (t,v,b,k)] per group:
    bias-MM:  lhsT = ones[2,128], rhs = (bias_hi|bias_lo)[2,512]  (start=True)
              -> u = 1 + csq_k  everywhere (hi/lo bf16 split for accuracy)
    per (t,v) chunk c = 4t+v, cols 32c..32c+32:
      MM1: lhsT = rhs1-chunk [128,128] (x^T as WEIGHTS), rhs = W1 = bd(-2c^T)
      MM2: lhsT = rhs2-chunk,                            rhs = W2 = bd(ones)
    => u[p, (t,v,b,k)] = 1 + csq + xsq - 2 x.c = 1 + d2   (fp32 accumulate)
  epilogue (all 128 partitions, k innermost in free dim):
    q = recip(u) (DVE approx) ; S = reduce_k q ; r = recip(S) ; qn = q * r
  store: qn[p, (t,v,b,k)] -> out[n, k]: 512B-contiguous runs per partition.
"""

import sys

sys.path.insert(0, "/opt/trn_rl_repo")

import numpy as np
from contextlib import ExitStack

import concourse.bass as bass
import concourse.bacc as bacc
import concourse.tile as tile
from concourse import mybir
from concourse.masks import make_identity
from concourse.tile_rust import add_dep_helper

FP32 = mybir.dt.float32
BF16 = mybir.dt.bfloat16

B, P, F, K = 8, 65536, 32, 8
NCORES = 8
G = 8          # pixel groups per core (8192 px each)
NT = 4         # load tiles per group
NV = 4         # 128-col transpose chunks per tile


def build_nc():
    # Bacc (not raw Bass): its compile() runs move_matmul_waits_to_ldweights
    # + generate_event_semaphores, legalizing instructions down to the 1
    # sync-wait the TRN2 ISA structs accept.
    nc = bacc.Bacc(name="clustering", trn_type="TRN2")

    x = nc.dram_tensor("x", [P, F], FP32, kind="ExternalInput")
    clusters = nc.dram_tensor("clusters", [K, F], FP32, kind="ExternalInput")
    out = nc.dram_tensor("out", [P, K], FP32, kind="ExternalOutput")
    scr_w1 = nc.dram_tensor("scr_w1", [F, K], BF16)

    # pixel n = 8192 g + 2048 t + 16 p + 4 v + b: each load partition reads
    # one fully contiguous 2 KiB run (16 consecutive pixel rows)
    x_r = x.rearrange("(g t p v b) f -> g t p v b f", g=G, t=NT, p=128, v=NV, b=4)
    # out free order (t, v, b, k): DRAM strides desc 64KiB/128B/32B/4B ->
    # (v,b,k) = 512B contiguous per partition
    out_r = out.rearrange("(g t p v b) k -> g p t v b k", g=G, t=NT, p=128, v=NV, b=4)

    with ExitStack() as ctx:
        tc = ctx.enter_context(tile.TileContext(nc))
        consts = ctx.enter_context(tc.tile_pool(name="consts", bufs=1))

        # NOTE: this walrus build allows at most ONE semaphore wait per PE
        # matmul (fused-LDW struct). Every tile consumed by a matmul is
        # routed through a single-engine producer ("wait collapser"), and a
        # throwaway PE op makes PE observe the constant producers early.

        # ---- constants ----
        id_bf = consts.tile([128, 128], BF16)
        make_identity(nc, id_bf)

        id8 = consts.tile([K, K], FP32)
        make_identity(nc, id8)

        ones2 = consts.tile([2, 128], BF16)
        nc.vector.memset(ones2, 1.0)



        # ---- cluster-derived weights ----
        c_dma = consts.tile([K, F], FP32)
        nc.sync.dma_start(out=c_dma, in_=clusters[:, :])
        # collapse the DMA wait onto Pool so the fp32 cT transpose (fused
        # LDW, single wait slot) sees one proc for both of its inputs
        c_sb = consts.tile([K, F], FP32)
        nc.gpsimd.tensor_copy(out=c_sb, in_=c_dma)

        # W2 = blockdiag(ones) [128, 32] bf16 (pure DVE memsets)
        W2 = consts.tile([128, 32], BF16)
        nc.vector.memset(W2, 0.0)
        for b in range(4):
            nc.vector.memset(W2[32 * b : 32 * b + 32, 8 * b : 8 * b + 8], 1.0)

        # cT = c^T at partitions 0..31 (walrus: matmul out must start at p0);
        # both inputs DVE-produced -> single wait on the transpose.
        # Pool stays OPEN so these banks are never reused (no reuse hazards
        # would fit in the single PE wait slot).
        spool = ctx.enter_context(tc.tile_pool(name="setup_psum", bufs=1, space="PSUM"))
        cT = spool.tile([F, K], FP32)
        nc.tensor.transpose(cT, c_sb, id8)
        cTm2 = consts.tile([F, K], BF16)
        nc.vector.tensor_scalar_mul(cTm2, cT, -2.0)

        # throwaway PE op: observe Pool (id_bf) early so the first real
        # x-transpose carries only its DMA wait
        warm = spool.tile([128, 128], BF16)
        nc.tensor.transpose(warm, id_bf, id_bf)

        # roundtrip through DRAM to place -2 c^T into the 4 diagonal blocks
        nc.sync.dma_start(out=scr_w1[:, :], in_=cTm2)
        W1d = consts.tile([128, 32], BF16)
        nc.vector.memset(W1d, 0.0)
        for b in range(4):
            nc.sync.dma_start(
                out=W1d[32 * b : 32 * b + 32, 8 * b : 8 * b + 8], in_=scr_w1[:, :]
            )
        # collapse the 4 DMA waits onto DVE, 2 at a time (DVE wait cap)
        W1 = consts.tile([128, 32], BF16)
        nc.vector.tensor_copy(W1[0:64, :], W1d[0:64, :])
        nc.vector.tensor_copy(W1[64:128, :], W1d[64:128, :])

        # bias = 1 + csq_k, split hi/lo into two bf16 rows for fp32-ish accuracy
        csq = consts.tile([K, F], FP32)
        nc.vector.tensor_mul(csq, c_sb, c_sb)
        bias_f32 = consts.tile([K, 1], FP32)
        nc.vector.tensor_reduce(
            bias_f32, csq, axis=mybir.AxisListType.X, op=mybir.AluOpType.add
        )
        nc.vector.tensor_scalar_add(bias_f32, bias_f32, 1.0)
        bias_hi_bf = consts.tile([K, 1], BF16)
        nc.vector.tensor_copy(bias_hi_bf, bias_f32)
        bias_lo_f32 = consts.tile([K, 1], FP32)
        # lo = bias - bf16(bias)
        nc.vector.tensor_tensor(
            out=bias_lo_f32, in0=bias_f32, in1=bias_hi_bf, op=mybir.AluOpType.subtract
        )
        # biasrows [2, 8] bf16 (row0 = hi, row1 = lo), built fully on-chip:
        # (hi|lo) pairs -> gpsimd copy (Pool-unify with id8) -> PE transpose
        # -> DVE copy-cast. The bias-MM reads it through a 64-rep step-0 AP.
        bias_hl = consts.tile([K, 2], FP32)
        nc.vector.tensor_copy(bias_hl[:, 0:1], bias_f32)
        nc.vector.tensor_copy(bias_hl[:, 1:2], bias_lo_f32)
        bias_hl_p = consts.tile([K, 2], FP32)
        nc.gpsimd.tensor_copy(out=bias_hl_p, in_=bias_hl)
        psum_b = spool.tile([2, K], FP32)
        nc.tensor.transpose(psum_b, bias_hl_p, id8)
        biasrows = consts.tile([2, K], BF16)
        nc.vector.tensor_copy(biasrows, psum_b)
        biasrows_bcast = bass.AP(
            tensor=biasrows.tensor,
            offset=biasrows.offset,
            ap=[biasrows.ap[0], [0, 64], [biasrows.ap[1][0], K]],
        )

        # ---- pipeline pools ----
        # one buffer per load (4 MiB total): no slot reuse -> the DIRECT2D
        # load DMAs (single wait slot) never carry recycle hazards
        xnat_p = ctx.enter_context(tc.tile_pool(name="xnat", bufs=G * NT))
        rhs_p = ctx.enter_context(tc.tile_pool(name="rhs", bufs=3))
        q_p = ctx.enter_context(tc.tile_pool(name="q", bufs=2))
        ps_xT = ctx.enter_context(tc.tile_pool(name="ps_xT", bufs=3, space="PSUM"))
        ps_u = ctx.enter_context(tc.tile_pool(name="ps_u", bufs=2, space="PSUM"))

        # Wait-budget discipline (this walrus: ~1 sync wait per PE
        # instruction; Tile splits each matmul into Ldweights + Matmult, and
        # the weights-input wait rides the Ldweights):
        #  - transposes: data (xnat) is the stationary operand -> its DMA
        #    wait lands on the LDW; the recycled-bank PE self-wait rides the
        #    Matmult. The ACT-copy release is pre-observed because MM1 of
        #    the previous tile waited on a newer ACT tick (rhs1 LDW).
        #  - chunk-MMs: rhs1/rhs2 are the stationary operands (ACT / DVE
        #    waits on their LDWs).
        #  - bias-MM: its weights are re-copied per group on DVE right
        #    after the previous group's recip, so the LDW's DVE wait
        #    transitively covers the PSUM-bank release; the Matmult keeps
        #    only the PE self-wait.

        prev_mm2 = [None]  # last chunk-MM2, to pin the PE stream order
        prev_recip = [None]  # previous group's u-recip, to gate ones2g copies

        def pe_after_prev(inst):
            if prev_mm2[0] is not None:
                add_dep_helper(inst.ins, prev_mm2[0].ins, True, "pe-stream-order")

        for g in range(G):
            psum_u = ps_u.tile([128, 512], FP32, tag="u", name="psu")
            # per-group copy of the bias-MM weights; pinned after the
            # previous group's recip on the same (DVE) proc so the bias-MM's
            # LDW wait transitively covers the PSUM bank release
            ones2g = consts.tile([2, 128], BF16, tag="ones2g", bufs=2, name="ones2g")
            cpy = nc.vector.tensor_copy(ones2g, ones2)
            if prev_recip[0] is not None:
                add_dep_helper(cpy.ins, prev_recip[0].ins, True, "dve-order-gate")
            # prime whole bank with bias: u = 1 + csq_k (start=True clears
            # has_written so the chunk-MMs accumulate onto the bias)
            bmm = nc.tensor.matmul(
                psum_u, ones2g, biasrows_bcast, start=True, stop=False,
                skip_group_check=True,
            )
            pe_after_prev(bmm)
            for t in range(NT):
                xnat = xnat_p.tile([128, 512], BF16, tag="xnat")
                # SWDGE cast-DMA fp32 -> bf16; 512B contiguous runs
                nc.gpsimd.dma_start(
                    out=xnat.rearrange("p (v b f) -> p v b f", v=NV, b=4),
                    in_=x_r[g, t],
                )

                psum_xT = ps_xT.tile([128, 1024], BF16, tag="xT", name="psxT")
                for v in range(NV):
                    tr = nc.tensor.transpose(
                        psum_xT[:, 128 * v : 128 * (v + 1)],
                        xnat[:, 128 * v : 128 * (v + 1)],
                        id_bf,
                    )
                    if v == 0:
                        pe_after_prev(tr)
                rhs1 = rhs_p.tile([128, 512], BF16, tag="rhs1")
                nc.scalar.copy(rhs1, psum_xT[:, 0:512])
                rhs2 = rhs_p.tile([128, 512], BF16, tag="rhs2")
                nc.vector.tensor_mul(rhs2, rhs1, rhs1)

                for v in range(NV):
                    c0 = 32 * (4 * t + v)
                    useg = psum_u[:, c0 : c0 + 32]
                    nc.tensor.matmul(
                        useg, rhs1[:, 128 * v : 128 * (v + 1)], W1,
                        start=False, stop=False, skip_group_check=True,
                    )
                    m2 = nc.tensor.matmul(
                        useg, rhs2[:, 128 * v : 128 * (v + 1)], W2,
                        start=False, stop=(v == NV - 1), skip_group_check=True,
                    )
                prev_mm2[0] = m2

            # ---- epilogue for 8192 pixels: [128, (v,t,b,k)], k innermost ----
            q_sb = q_p.tile([128, 512], FP32, tag="q")
            prev_recip[0] = nc.vector.reciprocal_approx_fast(out=q_sb, in_=psum_u)
            s_sb = q_p.tile([128, 64], FP32, tag="s")
            nc.vector.tensor_reduce(
                s_sb,
                q_sb.rearrange("p (c k) -> p c k", k=K),
                axis=mybir.AxisListType.X,
                op=mybir.AluOpType.add,
            )
            r_sb = q_p.tile([128, 64], FP32, tag="r")
            nc.vector.reciprocal_approx_fast(out=r_sb, in_=s_sb)
            qn = q_p.tile([128, 512], FP32, tag="qn")
            r_bcast = bass.AP(
                tensor=r_sb.tensor,
                offset=r_sb.offset,
                ap=[r_sb.ap[0], [r_sb.ap[1][0], 64], [0, K]],
            )
            nc.vector.tensor_tensor(
                out=qn,
                in0=q_sb,
                in1=r_bcast,
                op=mybir.AluOpType.mult,
            )
            # store: free order (t, v, b, k) -> 512B contiguous DRAM runs
            nc.sync.dma_start(
                out=out_r[g],
                in_=qn.rearrange("p (t v b k) -> p t v b k", t=NT, v=NV, b=4),
            )

    nc.compile()
    return nc


_NC = None


def _get_nc():
    global _NC
    if _NC is None:
        _NC = build_nc()
    return _NC


def kernel(x: np.ndarray, clusters: np.ndarray) -> np.ndarray:
    from concourse.bass_utils import run_bass_kernel_spmd

    x = np.ascontiguousarray(x, dtype=np.float32)
    clusters = np.ascontiguousarray(clusters, dtype=np.float32)
    assert x.shape == (B, P, F) and clusters.shape == (K, F)

    nc = _get_nc()
    in_maps = [{"x": x[i], "clusters": clusters} for i in range(NCORES)]
    res = run_bass_kernel_spmd(nc, in_maps, core_ids=list(range(NCORES)))
    return np.stack([res.results[i]["out"] for i in range(NCORES)], axis=0)


if __name__ == "__main__":
    rng = np.random.default_rng(0)
    x = rng.standard_normal((B, P, F), dtype=np.float32)
    c = rng.standard_normal((K, F), dtype=np.float32)
    got = kernel(x, c)
    print("out", got.shape, got.dtype, got[0, 0])
